# revision 56
# baseline (speedup 1.0000x reference)
"""Multi-head causal attention (B=2, S=2048, H=1024, 16 heads) on 8 TRN2
NeuronCores — v3 (no collectives).

Sharding: core c in 0..7 handles batch b = c // 4 and head group g = c % 4
(heads 4g..4g+3).  Each core computes Q/K/V projections for its 4 heads,
causal attention, and the PARTIAL out-projection (its 256 features through
the full Wo) for all 2048 rows.  Partials are written out in bf16 and the
HOST sums the 4 per-batch partials during unshard (row-parallel TP: the
unshard of partial shards is a sum).  No device collective at all: no
warmup barrier, no ReduceScatter, no exposed tail.

vs v2 (RS variant, 257us):
  - all inter-core communication removed; gpsimd/sync queues freed for DMA
  - host pre-tiles x strip-major ([NQS, 128, 8, QS]) so every DMA is
    contiguous 4KB-per-partition blocks (v2's strided rearrange produced
    1KB packets and ~1.5us dma_start issue cost each)
  - scalar engine carries ONLY the exp activations (v2 lost ~20us of
    scalar time to dma_start issue overhead)
  - diagonal score tiles only compute un-masked columns (v2 computed the
    full 512-wide strip and masked later)
  - out-projection bias moved to host (partials are summed there anyway)
"""

import sys

for _p in ("/opt/trn_rl_repo", "/root/.axon_site/_ro/trn_rl_repo"):
    if _p not in sys.path:
        sys.path.insert(0, _p)

import numpy as np

import concourse.bass as bass
import concourse.tile as tile
from concourse import bacc
import concourse.mybir as mybir

B = 2
S = 2048
HID = 1024
HPC = 4  # heads per core
DH = 64  # head dim
HG = HPC * DH  # 256: hidden slice per core
N_CORES = 8
GROUP = 4  # cores per batch (host-side reduction group)

F32 = mybir.dt.float32
BF = mybir.dt.bfloat16
F8 = mybir.dt.float8e4
AF = mybir.ActivationFunctionType
ALU = mybir.AluOpType
DR = mybir.MatmulPerfMode.DoubleRow

# Q/K path in fp8 (e4m3): weights are host-scaled by 8 (so all entries are
# fp8-normal), x is unscaled.  Q,K are kept scaled by 8 in SBUF and the
# whole 64x dequant plus the 1/sqrt(dh) folds into the exp scale.
QK_FP8 = True
EXP_SCALE = 1.0 / 512.0 if QK_FP8 else 1.0 / 8.0
XQK_DT = F8 if QK_FP8 else BF

KT = 128  # k tile (contraction positions per tile)
QS = 512  # q strip width
NQS = S // QS  # 4 q strips
NST = S // KT  # 16 k tiles


def build_nc():
    nc = bacc.Bacc(
        "TRN2", target_bir_lowering=False, debug=False, num_devices=N_CORES
    )

    # per-core inputs (sharded/tiled/bf16-cast by the host)
    # x tensors strip-major: [strip, partition, ktile, col]
    xq = nc.dram_tensor("xq", [NQS, 128, 8, QS], XQK_DT, kind="ExternalInput").ap()
    xk = nc.dram_tensor("xk", [NQS, 128, 8, QS], XQK_DT, kind="ExternalInput").ap()
    xv = nc.dram_tensor("xv", [NQS, 128, 8, QS], BF, kind="ExternalInput").ap()
    wq = nc.dram_tensor("wq", [128, 8, HG], XQK_DT, kind="ExternalInput").ap()
    wk = nc.dram_tensor("wk", [128, 8, HG], XQK_DT, kind="ExternalInput").ap()
    wv = nc.dram_tensor("wv", [128, 8, HG], BF, kind="ExternalInput").ap()
    w2 = nc.dram_tensor("w2", [128, 2, HID], BF, kind="ExternalInput").ap()
    bqv = nc.dram_tensor("bqv", [128, 2], F32, kind="ExternalInput").ap()
    # identity + strict-upper-triangular -1e5: the causal mask is applied by
    # accumulating id.T @ trineg into the diagonal psum block (PE work)
    # instead of a DVE multiply on the exp output
    idm = nc.dram_tensor("idm", [128, 128], BF, kind="ExternalInput").ap()
    trineg = nc.dram_tensor("trineg", [128, 128], BF, kind="ExternalInput").ap()

    # partial out-projection rows, bf16; host upcasts + sums the 4-core group
    out_part = nc.dram_tensor(
        "out_part", [NQS, QS, HID], BF, kind="ExternalOutput"
    ).ap()
    # last strip's pair-0 half-partial (the pair split lets it overlap the
    # pair-1 attention); host adds it into strip NQS-1
    out_extra = nc.dram_tensor(
        "out_extra", [QS, HID], BF, kind="ExternalOutput"
    ).ap()

    with tile.TileContext(nc) as tc:
        with (
            tc.tile_pool(name="wpool", bufs=1) as wpool,
            tc.tile_pool(name="qkv", bufs=1) as qkv,
            tc.tile_pool(name="xs", bufs=4) as xs,
            tc.tile_pool(name="atp", bufs=11) as atp,
            tc.tile_pool(name="atf", bufs=1) as atf,
            tc.tile_pool(name="otp", bufs=3) as otp,
            tc.tile_pool(name="osb", bufs=6) as osbp,
            tc.tile_pool(name="nrm", bufs=4) as nrm,
            tc.tile_pool(name="pbig", bufs=2, space="PSUM") as pbig,
            tc.tile_pool(name="ppso", bufs=2, space="PSUM") as ppso,
            tc.tile_pool(name="psml", bufs=2, space="PSUM") as psml,
        ):
            # ---- weights / constants ----
            # queue plan (DMA only on sync/gpsimd/scalar): sync carries
            # wq + all xq halves + w2 + even out stores; gpsimd carries
            # bq/wk + xk halves + late xv halves + odd out stores; scalar
            # carries tri/wv + the EARLY xv halves only (issued before the
            # first exp, so the exp stream owns the scalar engine after).
            wq_all = wpool.tile([128, 8, HG], XQK_DT, tag="wq")
            nc.sync.dma_start(wq_all[:], wq[:])
            wk_all = wpool.tile([128, 8, HG], XQK_DT, tag="wk")
            nc.gpsimd.dma_start(wk_all[:], wk[:])
            bq_sb = wpool.tile([128, 2], F32, tag="bq")
            nc.gpsimd.dma_start(bq_sb[:], bqv[:])
            id_sb = wpool.tile([128, 128], BF, tag="idm")
            nc.scalar.dma_start(id_sb[:], idm[:])
            tneg_sb = wpool.tile([128, 128], BF, tag="tneg")
            nc.scalar.dma_start(tneg_sb[:], trineg[:])
            wv_all = wpool.tile([128, 8, HG], BF, tag="wv")
            nc.scalar.dma_start(wv_all[:], wv[:])
            # ones row for the rowsum-broadcast outer-product matmul
            ones_sb = wpool.tile([1, DH], BF, tag="ones")
            nc.vector.memset(ones_sb[:], 1.0)
            # needed only from the first out-projection (~25us in)
            w2_all = wpool.tile([128, 2, HID], BF, tag="w2")

            # ---- persistent activations ----
            # QT/KT per (pair, strip): [dh', q] with heads 2p, 2p+1 in
            # partition halves
            qt_sb = [
                [
                    qkv.tile([128, QS], BF, tag=f"qt{p}{s}", name=f"qt{p}{s}")
                    for s in range(NQS)
                ]
                for p in range(2)
            ]
            kt_sb = [
                [
                    qkv.tile([128, QS], BF, tag=f"kt{p}{s}", name=f"kt{p}{s}")
                    for s in range(NQS)
                ]
                for p in range(2)
            ]
            # V natural [k, (head, dh+1)]: col DH of each head block is the
            # ones column (rowsums fall out of the attnV matmul, row DH)
            v_sb = [
                qkv.tile([128, HPC, DH + 1], BF, tag=f"v{st}", name=f"v{st}")
                for st in range(NST)
            ]
            for st in range(NST):
                nc.vector.memset(v_sb[st][:, :, DH : DH + 1], 1.0)

            # ---- projection steps for one strip (emitted lazily) ----
            # x loads are issued ~2 strips ahead; each strip split in two
            # half-loads (k-tiles 0-3 / 4-7) so the first matmuls start as
            # soon as the first half lands.
            x_tiles = {}

            def load_x(s):
                xt = {}
                # only strip-0 xv rides the scalar queue (startup-critical);
                # everything later keeps the exp stream unobstructed.  xv1
                # rides sync ahead of w2: it gates strip-1's attnV at ~30us
                # while gpsimd is still draining xk.
                xv_eng = (
                    nc.scalar
                    if s == 0
                    else (nc.sync if s in (1, 2) else nc.gpsimd)
                )
                for dram, tag, eng, dt in (
                    (xq, "xq", nc.sync, XQK_DT),
                    (xk, "xk", nc.gpsimd, XQK_DT),
                    (xv, "xv", xv_eng, BF),
                ):
                    halves = []
                    for h in range(2):
                        t = xs.tile(
                            [128, 4, QS], dt, tag=f"{tag}{h}", name=f"x{tag}{s}{h}"
                        )
                        if tag == "xv" and s == 0:
                            # strip-0 xv gates the very first attnV: split
                            # each half across two queues so all three DMA
                            # queues carry ~1MB of the startup-critical set
                            e2 = nc.sync if h == 0 else nc.gpsimd
                            eng.dma_start(
                                t[:, 0:2, :], dram[s, :, 4 * h : 4 * h + 2, :]
                            )
                            e2.dma_start(
                                t[:, 2:4, :], dram[s, :, 4 * h + 2 : 4 * h + 4, :]
                            )
                        else:
                            eng.dma_start(t[:], dram[s, :, 4 * h : 4 * h + 4, :])
                        halves.append(t)
                    xt[tag] = halves
                x_tiles[s] = xt

            def proj_steps(s):
                """Return a list of closures; each emits a small chunk of the
                strip-s projection work so it can interleave with attention
                of strip s-1.  load_x(s) must have been emitted earlier."""
                steps = []
                xt = x_tiles[s]

                # Q then K: k-contiguous accumulation into 2 m-tiles
                psq = {}

                if QK_FP8:
                    # fp8 DoubleRow: 2 k-tiles of contraction per matmul;
                    # operands laid out [128, 2, free] (pair along dim 1)
                    def qk_mm(tag, w_all, k2):
                        def f():
                            if k2 == 0:
                                psq[0] = psml.tile(
                                    [128, QS], F32, tag="sml", name=f"ps{tag}0"
                                )
                                psq[1] = psml.tile(
                                    [128, QS], F32, tag="sml", name=f"ps{tag}1"
                                )
                            kp = 2 * (k2 % 2)
                            for m in range(2):
                                nc.tensor.matmul(
                                    psq[m][:],
                                    w_all[
                                        :, 2 * k2 : 2 * k2 + 2,
                                        128 * m : 128 * m + 128,
                                    ],
                                    xt[tag][k2 // 2][:, kp : kp + 2, :],
                                    start=(k2 == 0),
                                    stop=(k2 == 3),
                                    perf_mode=DR,
                                )

                        return f

                    n_qk = 4
                else:
                    def qk_mm(tag, w_all, k):
                        def f():
                            if k == 0:
                                psq[0] = psml.tile(
                                    [128, QS], F32, tag="sml", name=f"ps{tag}0"
                                )
                                psq[1] = psml.tile(
                                    [128, QS], F32, tag="sml", name=f"ps{tag}1"
                                )
                            for m in range(2):
                                nc.tensor.matmul(
                                    psq[m][:],
                                    w_all[:, k, 128 * m : 128 * m + 128],
                                    xt[tag][k // 4][:, k % 4, :],
                                    start=(k == 0),
                                    stop=(k == 7),
                                )

                        return f

                    n_qk = 8

                def q_evac():
                    for m in range(2):
                        nc.vector.tensor_scalar_add(
                            qt_sb[m][s][:], psq[m][:], bq_sb[:, m : m + 1]
                        )

                def k_evac():
                    for m in range(2):
                        nc.vector.tensor_copy(kt_sb[m][s][:], psq[m][:])

                for k in range(n_qk):
                    steps.append(qk_mm("xq", wq_all, k))
                steps.append(q_evac)
                for k in range(n_qk):
                    steps.append(qk_mm("xk", wk_all, k))
                steps.append(k_evac)

                # V: 4 sequential 128-row sub-tiles, x-stationary
                psv = {}

                def v_mm(u, k2):
                    def f():
                        if k2 == 0:
                            psv[u] = psml.tile(
                                [128, QS], F32, tag="sml", name=f"psv{u}"
                            )
                        for k in (2 * k2, 2 * k2 + 1):
                            nc.tensor.matmul(
                                psv[u][:, 0:HG],
                                xt["xv"][k // 4][:, k % 4, 128 * u : 128 * u + 128],
                                wv_all[:, k, :],
                                start=(k == 0),
                                stop=(k == 7),
                            )

                    return f

                def v_evac(u):
                    def f():
                        st = 4 * s + u
                        nc.vector.tensor_copy(
                            v_sb[st][:, :, 0:DH],
                            psv[u][:, 0:HG].rearrange(
                                "p (h d) -> p h d", h=HPC
                            ),
                        )
                        del psv[u]

                    return f

                for u in range(4):
                    for k2 in range(4):
                        steps.append(v_mm(u, k2))
                    steps.append(v_evac(u))
                return steps

            # ---- partial out-projection for one strip ----
            # [512, 1024] partial rows from this core's 256 features ->
            # bf16 -> out_part (ExternalOutput).  Host sums the group.
            # `ps` selects which head-pairs to contract: the last strip is
            # emitted as two single-pair halves (pair 0 overlapping pair 1's
            # attention, into out_extra; host adds it in).
            def outproj_steps(s, ots, ps=(0, 1), dest=None, tail=False):
                steps = []
                po = {}

                def mm(u, eh):
                    def f():
                        if eh == 0:
                            po[0] = psml.tile(
                                [128, QS], F32, tag="sml", name="po0"
                            )
                            po[1] = psml.tile(
                                [128, QS], F32, tag="sml", name="po1"
                            )
                        for p in ps:
                            nc.tensor.matmul(
                                po[eh][:],
                                ots[p][:, 128 * u : 128 * u + 128],
                                w2_all[:, p, QS * eh : QS * eh + QS],
                                start=(p == ps[0]),
                                stop=(p == ps[-1]),
                            )

                    return f

                def evac(u):
                    def f():
                        ob = osbp.tile([128, 2 * QS], BF, tag="osb", name="osb")
                        for eh in range(2):
                            nc.vector.tensor_copy(
                                ob[:, QS * eh : QS * eh + QS], po[eh][:]
                            )
                        d = dest if dest is not None else out_part[s]
                        if tail:
                            # tail half: nothing overlaps the stores, so
                            # spread them over three queues (exp is done,
                            # scalar is free) and split in halves
                            engs = (nc.sync, nc.gpsimd, nc.scalar)
                            for eh in range(2):
                                engs[(2 * u + eh) % 3].dma_start(
                                    d[
                                        128 * u : 128 * u + 128,
                                        QS * eh : QS * eh + QS,
                                    ],
                                    ob[:, QS * eh : QS * eh + QS],
                                )
                        else:
                            eng = nc.sync if u % 2 == 0 else nc.gpsimd
                            eng.dma_start(d[128 * u : 128 * u + 128], ob[:])

                    return f

                for u in range(4):
                    steps.append(mm(u, 0))
                    steps.append(mm(u, 1))
                    steps.append(evac(u))
                return steps

            # ---- pre-pulled scores+exp ----
            # the exp stream saturates the scalar engine in the late strips
            # (strip 3 has 27us of exp vs ~23us of PE); pull the first few
            # k-tiles' scores+exp of strip s into strip s-1, where the
            # scalar engine has slack.  The at tiles persist in dedicated
            # one-shot slots until the owning strip's attnV consumes them.
            ats_pre = {}
            PULLS = {1: (0, 1, 2, 3), 2: (0, 1, 2, 3), 3: (0, 1, 2, 3, 4, 5)}

            def pre_scores(s2, p, j):
                def f():
                    psc = pbig.tile(
                        [128, 2 * QS], F32, tag="big", name="pscp"
                    )
                    for hh in range(2):
                        hp = 64 * hh
                        nc.tensor.matmul(
                            psc[:, QS * hh : QS * hh + QS],
                            kt_sb[p][j // 4][
                                hp : hp + 64,
                                128 * (j % 4) : 128 * (j % 4) + 128,
                            ],
                            qt_sb[p][s2][hp : hp + 64, :],
                            start=True,
                            stop=True,
                        )
                    at = atf.tile(
                        [128, 2 * QS], BF, tag=f"pre{s2 % 2}{p}{j}",
                        name=f"pre{s2}{p}{j}",
                    )
                    nc.scalar.activation(at[:], psc[:], AF.Exp, scale=EXP_SCALE)
                    ats_pre[(s2, p, j)] = at

                return f

            # ---- main pipeline ----
            load_x(0)
            load_x(1)
            nc.sync.dma_start(w2_all[:], w2[:])
            # strip 0: emit only the Q/K projection now.  The V matmuls
            # wait on the xv0 DMA (~11us) and would block the in-order PE
            # queue ahead of strip-0's first scores; they are emitted after
            # the scores prologue instead.
            p0_steps = proj_steps(0)
            n_qk0 = (4 if QK_FP8 else 8) * 2 + 2
            for st in p0_steps[:n_qk0]:
                st()
            v0_steps = p0_steps[n_qk0:]

            pending = []

            def pump(n):
                for _ in range(min(n, len(pending))):
                    pending.pop(0)()

            for s in range(NQS):
                if s + 2 < NQS:
                    pending.append(lambda s2=s + 2: load_x(s2))
                if s + 1 < NQS:
                    ps_list = proj_steps(s + 1)
                    pulls = [
                        pre_scores(s + 1, p2, j)
                        for j in PULLS.get(s + 1, ())
                        for p2 in range(2)
                    ]
                    # splice the pulled scores+exp evenly through the proj
                    # steps (all after q_evac, which they depend on) so the
                    # exp stream spreads across the strip instead of
                    # bunching at its end
                    n_qk = 4 if QK_FP8 else 8
                    qe = n_qk + 1  # index just past q_evac
                    body = ps_list[qe:]
                    merged = ps_list[:qe]
                    if pulls:
                        step = max(1, len(body) // len(pulls))
                        bi = 0
                        for k, pu in enumerate(pulls):
                            nxt = min(len(body), (k + 1) * step)
                            merged += body[bi:nxt]
                            merged.append(pu)
                            bi = nxt
                        merged += body[bi:]
                    else:
                        merged += body
                    pending += merged
                jmax = 4 * s + 4
                # interleave budget: spread pending steps over this strip's
                # (pair, j) iterations, skipping the first few so the
                # attention front isn't stalled by not-yet-landed x DMAs
                skip = 3 if s == 0 else 2
                iters = 2 * (jmax + 2) - skip
                rate = (len(pending) + 2 + iters - 1) // iters
                it_ctr = [0]

                ot = [
                    otp.tile([128, QS], BF, tag=f"ot{p}", name=f"ot{p}")
                    for p in range(2)
                ]
                for p in range(2):
                    pso = {}
                    ats = {}

                    def do_scores(j):
                        pre = ats_pre.pop((s, p, j), None)
                        if pre is not None:
                            ats[j] = pre
                            return
                        psc = pbig.tile(
                            [128, 2 * QS], F32, tag="big", name="psc"
                        )
                        i = j - 4 * s
                        # diagonal tiles: columns < 128*i are fully masked --
                        # don't even compute them
                        off = 128 * i if j >= 4 * s else 0
                        for hh in range(2):
                            hp = 64 * hh
                            nc.tensor.matmul(
                                psc[:, QS * hh + off : QS * hh + QS],
                                kt_sb[p][j // 4][
                                    hp : hp + 64, 128 * (j % 4) : 128 * (j % 4) + 128
                                ],
                                qt_sb[p][s][hp : hp + 64, off:QS],
                                start=True,
                                stop=True,
                            )
                        if j >= 4 * s:
                            # causal mask: accumulate -1e5 onto the masked
                            # cells of the 128-wide diagonal block (id.T @
                            # trineg = trineg), so exp zeroes them with no
                            # DVE op in the attnV critical path
                            for hh in range(2):
                                o = QS * hh + off
                                nc.tensor.matmul(
                                    psc[:, o : o + 128],
                                    id_sb[:],
                                    tneg_sb[:],
                                    start=False,
                                    stop=True,
                                    skip_group_check=True,
                                )
                        at = atp.tile([128, 2 * QS], BF, tag="at", name="at")
                        if off == 0:
                            nc.scalar.activation(
                                at[:], psc[:], AF.Exp, scale=EXP_SCALE
                            )
                        else:
                            # one strided ACT covers both head-halves'
                            # unmasked columns
                            sl = (
                                lambda t: t.rearrange(
                                    "pp (h c) -> pp h c", h=2
                                )[:, :, off:QS]
                            )
                            nc.scalar.activation(
                                sl(at[:]), sl(psc[:]), AF.Exp,
                                scale=EXP_SCALE,
                            )
                        ats[j] = at

                    def attn_v(hh, j):
                        off = 128 * (j - 4 * s) if j >= 4 * s else 0
                        if j == 0:
                            pso[hh] = ppso.tile(
                                [DH + 1, QS], F32, tag="pso", name=f"pso{hh}"
                            )
                        nc.tensor.matmul(
                            pso[hh][:, off:QS],
                            v_sb[j][:, 2 * p + hh, :],
                            ats[j][:, QS * hh + off : QS * hh + QS],
                            start=(j == 0),
                            stop=(j == jmax - 1),
                        )

                    # normalize: rowsum (psum row DH) -> reciprocal ->
                    # multiply into the persistent OT tile.  Split in two:
                    # the pre half only issues the DVE rowsum copy; the PE
                    # half (rbc broadcast) is emitted later with PE filler
                    # in between so the in-order PE queue never waits on
                    # the DVE queue draining (that wait was >3.4us and
                    # re-throttled the HAM clock at every strip boundary).
                    rss = {}

                    def normalize_pre(hh):
                        rs = nrm.tile([1, QS], BF, tag="rs", name="rs")
                        nc.vector.tensor_copy(rs[:], pso[hh][DH : DH + 1])
                        rss[hh] = rs

                    def normalize_post(hh):
                        rbc = pbig.tile([64, QS], F32, tag="big", name="rbc")
                        nc.tensor.matmul(
                            rbc[:], ones_sb[:], rss[hh][:], start=True,
                            stop=True,
                        )
                        rrec = nrm.tile([64, QS], F32, tag="rrec", name="rrec")
                        nc.vector.reciprocal_approx_fast(rrec[:], rbc[:])
                        nc.vector.tensor_tensor(
                            ot[p][64 * hh : 64 * hh + 64],
                            pso[hh][0:DH],
                            rrec[:],
                            ALU.mult,
                        )

                    # hh1's attnV stream lags hh0 by two k-tiles so each
                    # head-half's normalize chain overlaps remaining matmuls
                    # and its pso bank frees before the next pair needs it
                    # scores cursor: keep TWO not-yet-pulled score tiles in
                    # flight ahead of attnV.  Pulled tiles are free dict
                    # hits, so during a pulled window the cursor streams
                    # real scores (and their exps) ahead -- this is what
                    # keeps the scalar engine fed at strip starts.
                    npull = sum(1 for jj in PULLS.get(s, ()) if jj < jmax)
                    cur = [0]

                    def emit_scores_to(k):
                        while cur[0] < min(k, jmax):
                            do_scores(cur[0])
                            cur[0] += 1

                    emit_scores_to(npull + 2)
                    if s == 0 and p == 0:
                        # the first attnV waits ~15us for the xv0 DMA; the
                        # in-order PE queue would idle behind it, so slot
                        # strip-1's Q/K projection matmuls and the deferred
                        # strip-0 V projection in ahead of it
                        pump(12)
                        for st in v0_steps:
                            st()
                    for j in range(jmax + 2):
                        emit_scores_to(j + 3)
                        it_ctr[0] += 1
                        if it_ctr[0] > skip:
                            pump(rate)
                        if j < jmax:
                            attn_v(0, j)
                            if j == jmax - 1:
                                normalize_pre(0)
                        if j >= 2:
                            attn_v(1, j - 2)
                            if j - 2 == jmax - 1:
                                normalize_pre(1)
                            del ats[j - 2]
                        if j == jmax:
                            normalize_post(0)
                    if p == 1:
                        pump(len(pending))
                    normalize_post(1)
                    if s == NQS - 1 and p == 0:
                        # last strip: pair-0's half of the out-projection
                        # goes through `pending` so it interleaves with
                        # pair-1's attention instead of sitting in the tail
                        pending.extend(
                            outproj_steps(s, ot, ps=(0,), dest=out_extra)
                        )
                # out-projection of this strip: route it through `pending`
                # so its matmuls AND its DVE casts spread across the next
                # strip's attention instead of forming a block at the strip
                # boundary (the DVE FIFO block was stalling the PE)
                if s == NQS - 1:
                    pump(len(pending))
                    for st in outproj_steps(
                        s, ot, ps=(1,), dest=out_part[s], tail=True
                    ):
                        st()
                else:
                    pending.extend(outproj_steps(s, ot))

    nc.compile()
    return nc


_NC = None
_RUNNER = None


def _get_runner():
    """Build the compiled 8-core PJRT callable once and cache it."""
    global _NC, _RUNNER
    if _RUNNER is not None:
        return _RUNNER

    import jax
    import numpy as _np
    from jax.sharding import Mesh, PartitionSpec
    from jax.experimental.shard_map import shard_map
    from concourse.bass2jax import (
        _bass_exec_p,
        install_neuronx_cc_hook,
        partition_id_tensor,
    )

    _NC = build_nc()
    nc = _NC
    install_neuronx_cc_hook()

    partition_name = nc.partition_id_tensor.name if nc.partition_id_tensor else None
    in_names = []
    out_names = []
    out_avals = []
    zero_outs = []
    for alloc in nc.m.functions[0].allocations:
        if not isinstance(alloc, mybir.MemoryLocationSet):
            continue
        name = alloc.memorylocations[0].name
        if alloc.kind == "ExternalInput":
            if name != partition_name:
                in_names.append(name)
        elif alloc.kind == "ExternalOutput":
            shape = tuple(alloc.tensor_shape)
            dtype = mybir.dt.np(alloc.dtype)
            out_names.append(name)
            out_avals.append(jax.core.ShapedArray(shape, dtype))
            zero_outs.append(_np.zeros(shape, dtype))
    n_params = len(in_names)
    n_outs = len(out_avals)
    all_in_names = list(in_names) + list(out_names)
    if partition_name is not None:
        all_in_names.append(partition_name)

    def _body(*args):
        operands = list(args)
        if partition_name is not None:
            operands.append(partition_id_tensor())
        outs = _bass_exec_p.bind(
            *operands,
            out_avals=tuple(out_avals),
            in_names=tuple(all_in_names),
            out_names=tuple(out_names),
            lowering_input_output_aliases=(),
            sim_require_finite=True,
            sim_require_nnan=True,
            nc=nc,
        )
        return tuple(outs)

    devices = jax.devices()[:N_CORES]
    mesh = Mesh(np.asarray(devices), ("core",))
    in_specs = (PartitionSpec("core"),) * (n_params + n_outs)
    out_specs = (PartitionSpec("core"),) * n_outs
    sharded = jax.jit(
        shard_map(
            _body, mesh=mesh, in_specs=in_specs, out_specs=out_specs, check_rep=False
        ),
        keep_unused=True,
    )

    def run(in_maps):
        per_core = [[_np.asarray(m[name]) for name in in_names] for m in in_maps]
        concat_in = [
            _np.concatenate([per_core[c][i] for c in range(N_CORES)], axis=0)
            for i in range(n_params)
        ]
        concat_zeros = [
            _np.zeros((N_CORES * z.shape[0], *z.shape[1:]), z.dtype)
            for z in zero_outs
        ]
        out_arrs = sharded(*concat_in, *concat_zeros)
        return [
            {
                name: _np.asarray(out_arrs[i]).reshape(
                    N_CORES, *out_avals[i].shape
                )[c]
                for i, name in enumerate(out_names)
            }
            for c in range(N_CORES)
        ]

    _RUNNER = run
    return run


_BO_EFF = None


def make_in_maps(query, key, value, Wq, bq, Wk, bk, Wv, bv, Wo, bo):
    global _BO_EFF
    from ml_dtypes import bfloat16, float8_e4m3

    query = np.asarray(query, dtype=np.float32)
    key = np.asarray(key, dtype=np.float32)
    value = np.asarray(value, dtype=np.float32)
    Wq = np.asarray(Wq, dtype=np.float32)
    bq = np.asarray(bq, dtype=np.float32)
    Wk = np.asarray(Wk, dtype=np.float32)
    Wv = np.asarray(Wv, dtype=np.float32)
    bv = np.asarray(bv, dtype=np.float32)
    Wo = np.asarray(Wo, dtype=np.float32)
    bo = np.asarray(bo, dtype=np.float32)

    # K bias is softmax-invariant (constant per q row) -> dropped.
    # V bias: softmax rows sum to 1, so it contributes bv @ Wo.T -> fold
    # into the output bias, added on host during assembly.
    _BO_EFF = bo + bv @ Wo.T

    xqk_np = float8_e4m3 if QK_FP8 else bfloat16
    # Wq/Wk entries are ~U(-1/32, 1/32) -- scale by 8 so every value is
    # fp8-normal; Q,K come out scaled by 8 and exp's scale absorbs it
    wqk_scale = 8.0 if QK_FP8 else 1.0

    # x strip-major: xt[s, p, t, c] = x[512 s + c, 128 t + p]
    def tile_x(x, dt):  # [S, HID] -> [NQS, 128, 8, QS]
        t = x.reshape(NQS, QS, 8, 128).transpose(0, 3, 2, 1)
        return np.ascontiguousarray(t).astype(dt)

    xqs = [tile_x(query[b], xqk_np) for b in range(B)]
    xks = [tile_x(key[b], xqk_np) for b in range(B)]
    xvs = [tile_x(value[b], bfloat16) for b in range(B)]

    # causal-mask helpers for the diagonal 128x128 block: identity (matmul
    # lhsT) and -1e5 on strictly-masked cells [k, q] (q < k)
    idm = np.ascontiguousarray(np.eye(128, dtype=bfloat16))
    trineg = np.ascontiguousarray(
        np.where(
            np.arange(128)[None, :] < np.arange(128)[:, None], -1.0e5, 0.0
        ).astype(bfloat16)
    )

    def tile_w(wT, dt):  # [HID, F] (= W[hsl].T) -> [128, 8, F]
        t = wT.reshape(8, 128, -1).transpose(1, 0, 2)
        return np.ascontiguousarray(t).astype(dt)

    in_maps = []
    for c in range(N_CORES):
        b = c // GROUP
        g = c % GROUP
        hsl = slice(HG * g, HG * g + HG)
        wq_g = tile_w(Wq[hsl].T * wqk_scale, xqk_np)  # [128, 8, 256]
        wk_g = tile_w(Wk[hsl].T * wqk_scale, xqk_np)
        wv_g = tile_w(Wv[hsl].T, bfloat16)
        w2_t = Wo[:, hsl].T.reshape(2, 128, HID).transpose(1, 0, 2)
        w2_g = np.ascontiguousarray(w2_t).astype(bfloat16)  # [128, 2, 1024]
        # Q bias rides on the 8x-scaled Q
        bq_g = np.ascontiguousarray(
            bq[hsl].reshape(2, 128).T * wqk_scale
        ).astype(np.float32)
        in_maps.append(
            {
                "xq": xqs[b],
                "xk": xks[b],
                "xv": xvs[b],
                "wq": wq_g,
                "wk": wk_g,
                "wv": wv_g,
                "w2": w2_g,
                "bqv": bq_g,
                "idm": idm,
                "trineg": trineg,
            }
        )
    return in_maps


def assemble_output(results):
    # core group {4b..4b+3} holds bf16 partial out-projections of batch b;
    # sum them (the row-parallel TP unshard) and add the folded bias
    out = np.empty((B, S, HID), dtype=np.float32)
    for b in range(B):
        acc = results[GROUP * b]["out_part"].astype(np.float32)
        acc[NQS - 1] += results[GROUP * b]["out_extra"].astype(np.float32)
        for r in range(1, GROUP):
            res = results[GROUP * b + r]
            acc = acc + res["out_part"].astype(np.float32)
            acc[NQS - 1] += res["out_extra"].astype(np.float32)
        out[b] = acc.reshape(S, HID)
    out += _BO_EFF
    return out


def kernel(**inputs) -> np.ndarray:
    in_maps = make_in_maps(**inputs)
    run = _get_runner()
    results = run(in_maps)
    return assemble_output(results)


if __name__ == "__main__":
    import reference

    inputs = {k: np.asarray(v) for k, v in reference.setup_inputs().items()}
    got = kernel(**inputs)
    want = np.asarray(reference.reference(**inputs))
    err = np.linalg.norm(got - want) / np.linalg.norm(want)
    print("Relative error:", err)


# revision 58
# speedup vs baseline: 1.0162x; 1.0162x over previous
"""Multi-head causal attention (B=2, S=2048, H=1024, 16 heads) on 8 TRN2
NeuronCores — v3 (no collectives).

Sharding: core c in 0..7 handles batch b = c // 4 and head group g = c % 4
(heads 4g..4g+3).  Each core computes Q/K/V projections for its 4 heads,
causal attention, and the PARTIAL out-projection (its 256 features through
the full Wo) for all 2048 rows.  Partials are written out in bf16 and the
HOST sums the 4 per-batch partials during unshard (row-parallel TP: the
unshard of partial shards is a sum).  No device collective at all: no
warmup barrier, no ReduceScatter, no exposed tail.

vs v2 (RS variant, 257us):
  - all inter-core communication removed; gpsimd/sync queues freed for DMA
  - host pre-tiles x strip-major ([NQS, 128, 8, QS]) so every DMA is
    contiguous 4KB-per-partition blocks (v2's strided rearrange produced
    1KB packets and ~1.5us dma_start issue cost each)
  - scalar engine carries ONLY the exp activations (v2 lost ~20us of
    scalar time to dma_start issue overhead)
  - diagonal score tiles only compute un-masked columns (v2 computed the
    full 512-wide strip and masked later)
  - out-projection bias moved to host (partials are summed there anyway)
"""

import sys

for _p in ("/opt/trn_rl_repo", "/root/.axon_site/_ro/trn_rl_repo"):
    if _p not in sys.path:
        sys.path.insert(0, _p)

import numpy as np

import concourse.bass as bass
import concourse.tile as tile
from concourse import bacc
import concourse.mybir as mybir

B = 2
S = 2048
HID = 1024
HPC = 4  # heads per core
DH = 64  # head dim
HG = HPC * DH  # 256: hidden slice per core
N_CORES = 8
GROUP = 4  # cores per batch (host-side reduction group)

F32 = mybir.dt.float32
BF = mybir.dt.bfloat16
F8 = mybir.dt.float8e4
AF = mybir.ActivationFunctionType
ALU = mybir.AluOpType
DR = mybir.MatmulPerfMode.DoubleRow

# Q/K path in fp8 (e4m3): weights are host-scaled by 8 (so all entries are
# fp8-normal), x is unscaled.  Q,K are kept scaled by 8 in SBUF and the
# whole 64x dequant plus the 1/sqrt(dh) folds into the exp scale.
QK_FP8 = True
EXP_SCALE = 1.0 / 512.0 if QK_FP8 else 1.0 / 8.0
XQK_DT = F8 if QK_FP8 else BF

KT = 128  # k tile (contraction positions per tile)
QS = 512  # q strip width
NQS = S // QS  # 4 q strips
NST = S // KT  # 16 k tiles


def build_nc():
    nc = bacc.Bacc(
        "TRN2", target_bir_lowering=False, debug=False, num_devices=N_CORES
    )

    # per-core inputs (sharded/tiled/bf16-cast by the host)
    # x tensors strip-major: [strip, partition, ktile, col]
    xq = nc.dram_tensor("xq", [NQS, 128, 8, QS], XQK_DT, kind="ExternalInput").ap()
    xk = nc.dram_tensor("xk", [NQS, 128, 8, QS], XQK_DT, kind="ExternalInput").ap()
    xv = nc.dram_tensor("xv", [NQS, 128, 8, QS], BF, kind="ExternalInput").ap()
    wq = nc.dram_tensor("wq", [128, 8, HG], XQK_DT, kind="ExternalInput").ap()
    wk = nc.dram_tensor("wk", [128, 8, HG], XQK_DT, kind="ExternalInput").ap()
    wv = nc.dram_tensor("wv", [128, 8, HG], BF, kind="ExternalInput").ap()
    w2 = nc.dram_tensor("w2", [128, 2, HID], BF, kind="ExternalInput").ap()
    bqv = nc.dram_tensor("bqv", [128, 2], F32, kind="ExternalInput").ap()
    # identity + strict-upper-triangular -1e5: the causal mask is applied by
    # accumulating id.T @ trineg into the diagonal psum block (PE work)
    # instead of a DVE multiply on the exp output
    idm = nc.dram_tensor("idm", [128, 128], BF, kind="ExternalInput").ap()
    trineg = nc.dram_tensor("trineg", [128, 128], BF, kind="ExternalInput").ap()

    # partial out-projection rows, bf16; host upcasts + sums the 4-core group
    out_part = nc.dram_tensor(
        "out_part", [NQS, QS, HID], BF, kind="ExternalOutput"
    ).ap()
    # last strip's pair-0 half-partial (the pair split lets it overlap the
    # pair-1 attention); host adds it into strip NQS-1
    out_extra = nc.dram_tensor(
        "out_extra", [QS, HID], BF, kind="ExternalOutput"
    ).ap()

    with tile.TileContext(nc) as tc:
        with (
            tc.tile_pool(name="wpool", bufs=1) as wpool,
            tc.tile_pool(name="qkv", bufs=1) as qkv,
            tc.tile_pool(name="xs", bufs=4) as xs,
            tc.tile_pool(name="atp", bufs=9) as atp,
            tc.tile_pool(name="atf", bufs=1) as atf,
            tc.tile_pool(name="otp", bufs=3) as otp,
            tc.tile_pool(name="osb", bufs=4) as osbp,
            tc.tile_pool(name="nrm", bufs=4) as nrm,
            tc.tile_pool(name="pbig", bufs=2, space="PSUM") as pbig,
            tc.tile_pool(name="ppso", bufs=2, space="PSUM") as ppso,
            tc.tile_pool(name="psml", bufs=2, space="PSUM") as psml,
        ):
            # ---- weights / constants ----
            # queue plan (DMA only on sync/gpsimd/scalar): sync carries
            # wq + all xq halves + w2 + even out stores; gpsimd carries
            # bq/wk + xk halves + late xv halves + odd out stores; scalar
            # carries tri/wv + the EARLY xv halves only (issued before the
            # first exp, so the exp stream owns the scalar engine after).
            bq_sb = wpool.tile([128, 2], F32, tag="bq")
            nc.gpsimd.dma_start(bq_sb[:], bqv[:])
            wq_all = wpool.tile([128, 8, HG], XQK_DT, tag="wq")
            nc.sync.dma_start(wq_all[:], wq[:])
            wk_all = wpool.tile([128, 8, HG], XQK_DT, tag="wk")
            nc.gpsimd.dma_start(wk_all[:], wk[:])
            id_sb = wpool.tile([128, 128], BF, tag="idm")
            nc.scalar.dma_start(id_sb[:], idm[:])
            tneg_sb = wpool.tile([128, 128], BF, tag="tneg")
            nc.scalar.dma_start(tneg_sb[:], trineg[:])
            wv_all = wpool.tile([128, 8, HG], BF, tag="wv")
            nc.scalar.dma_start(wv_all[:], wv[:])
            # ones row for the rowsum-broadcast outer-product matmul
            ones_sb = wpool.tile([1, DH], BF, tag="ones")
            nc.vector.memset(ones_sb[:], 1.0)
            # needed only from the first out-projection (~25us in)
            w2_all = wpool.tile([128, 2, HID], BF, tag="w2")

            # ---- persistent activations ----
            # QT/KT per (pair, strip): [dh', q] with heads 2p, 2p+1 in
            # partition halves
            qt_sb = [
                [
                    qkv.tile([128, QS], BF, tag=f"qt{p}{s}", name=f"qt{p}{s}")
                    for s in range(NQS)
                ]
                for p in range(2)
            ]
            kt_sb = [
                [
                    qkv.tile([128, QS], BF, tag=f"kt{p}{s}", name=f"kt{p}{s}")
                    for s in range(NQS)
                ]
                for p in range(2)
            ]
            # V natural [k, (head, dh+1)]: col DH of each head block is the
            # ones column (rowsums fall out of the attnV matmul, row DH)
            v_sb = [
                qkv.tile([128, HPC, DH + 1], BF, tag=f"v{st}", name=f"v{st}")
                for st in range(NST)
            ]
            for st in range(NST):
                nc.vector.memset(v_sb[st][:, :, DH : DH + 1], 1.0)

            # ---- projection steps for one strip (emitted lazily) ----
            # x loads are issued ~2 strips ahead; each strip split in two
            # half-loads (k-tiles 0-3 / 4-7) so the first matmuls start as
            # soon as the first half lands.
            x_tiles = {}

            def load_x(s):
                xt = {}
                # only strip-0 xv rides the scalar queue (startup-critical);
                # everything later keeps the exp stream unobstructed.  xv1
                # rides sync ahead of w2: it gates strip-1's attnV at ~30us
                # while gpsimd is still draining xk.
                xv_eng = (
                    nc.scalar
                    if s == 0
                    else (nc.sync if s in (1, 2) else nc.gpsimd)
                )
                for dram, tag, eng, dt in (
                    (xq, "xq", nc.sync, XQK_DT),
                    (xk, "xk", nc.gpsimd, XQK_DT),
                    (xv, "xv", xv_eng, BF),
                ):
                    halves = []
                    for h in range(2):
                        t = xs.tile(
                            [128, 4, QS], dt, tag=f"{tag}{h}", name=f"x{tag}{s}{h}"
                        )
                        if tag == "xv" and s == 0:
                            # strip-0 xv gates the very first attnV: split
                            # each half across two queues so all three DMA
                            # queues carry ~1MB of the startup-critical set
                            e2 = nc.sync if h == 0 else nc.gpsimd
                            eng.dma_start(
                                t[:, 0:2, :], dram[s, :, 4 * h : 4 * h + 2, :]
                            )
                            e2.dma_start(
                                t[:, 2:4, :], dram[s, :, 4 * h + 2 : 4 * h + 4, :]
                            )
                        else:
                            eng.dma_start(t[:], dram[s, :, 4 * h : 4 * h + 4, :])
                        halves.append(t)
                    xt[tag] = halves
                x_tiles[s] = xt

            def proj_steps(s):
                """Return a list of closures; each emits a small chunk of the
                strip-s projection work so it can interleave with attention
                of strip s-1.  load_x(s) must have been emitted earlier."""
                steps = []
                xt = x_tiles[s]

                # Q then K: k-contiguous accumulation into 2 m-tiles
                psq = {}

                if QK_FP8:
                    # fp8 DoubleRow: 2 k-tiles of contraction per matmul;
                    # operands laid out [128, 2, free] (pair along dim 1)
                    def qk_mm(tag, w_all, k2):
                        def f():
                            if k2 == 0:
                                psq[0] = psml.tile(
                                    [128, QS], F32, tag="sml", name=f"ps{tag}0"
                                )
                                psq[1] = psml.tile(
                                    [128, QS], F32, tag="sml", name=f"ps{tag}1"
                                )
                            kp = 2 * (k2 % 2)
                            for m in range(2):
                                nc.tensor.matmul(
                                    psq[m][:],
                                    w_all[
                                        :, 2 * k2 : 2 * k2 + 2,
                                        128 * m : 128 * m + 128,
                                    ],
                                    xt[tag][k2 // 2][:, kp : kp + 2, :],
                                    start=(k2 == 0),
                                    stop=(k2 == 3),
                                    perf_mode=DR,
                                )

                        return f

                    n_qk = 4
                else:
                    def qk_mm(tag, w_all, k):
                        def f():
                            if k == 0:
                                psq[0] = psml.tile(
                                    [128, QS], F32, tag="sml", name=f"ps{tag}0"
                                )
                                psq[1] = psml.tile(
                                    [128, QS], F32, tag="sml", name=f"ps{tag}1"
                                )
                            for m in range(2):
                                nc.tensor.matmul(
                                    psq[m][:],
                                    w_all[:, k, 128 * m : 128 * m + 128],
                                    xt[tag][k // 4][:, k % 4, :],
                                    start=(k == 0),
                                    stop=(k == 7),
                                )

                        return f

                    n_qk = 8

                def q_evac():
                    for m in range(2):
                        nc.vector.tensor_scalar_add(
                            qt_sb[m][s][:], psq[m][:], bq_sb[:, m : m + 1]
                        )

                def k_evac():
                    for m in range(2):
                        nc.vector.tensor_copy(kt_sb[m][s][:], psq[m][:])

                for k in range(n_qk):
                    steps.append(qk_mm("xq", wq_all, k))
                steps.append(q_evac)
                for k in range(n_qk):
                    steps.append(qk_mm("xk", wk_all, k))
                steps.append(k_evac)

                # V: 4 sequential 128-row sub-tiles, x-stationary
                psv = {}

                def v_mm(u, k2):
                    def f():
                        if k2 == 0:
                            psv[u] = psml.tile(
                                [128, QS], F32, tag="sml", name=f"psv{u}"
                            )
                        for k in (2 * k2, 2 * k2 + 1):
                            nc.tensor.matmul(
                                psv[u][:, 0:HG],
                                xt["xv"][k // 4][:, k % 4, 128 * u : 128 * u + 128],
                                wv_all[:, k, :],
                                start=(k == 0),
                                stop=(k == 7),
                            )

                    return f

                def v_evac(u):
                    def f():
                        st = 4 * s + u
                        nc.vector.tensor_copy(
                            v_sb[st][:, :, 0:DH],
                            psv[u][:, 0:HG].rearrange(
                                "p (h d) -> p h d", h=HPC
                            ),
                        )
                        del psv[u]

                    return f

                for u in range(4):
                    for k2 in range(4):
                        steps.append(v_mm(u, k2))
                    steps.append(v_evac(u))
                return steps

            # ---- partial out-projection for one strip ----
            # [512, 1024] partial rows from this core's 256 features ->
            # bf16 -> out_part (ExternalOutput).  Host sums the group.
            # `ps` selects which head-pairs to contract: the last strip is
            # emitted as two single-pair halves (pair 0 overlapping pair 1's
            # attention, into out_extra; host adds it in).
            def outproj_steps(s, ots, ps=(0, 1), dest=None, tail=False):
                steps = []
                po = {}

                def mm(u, eh):
                    def f():
                        if eh == 0:
                            po[0] = psml.tile(
                                [128, QS], F32, tag="sml", name="po0"
                            )
                            po[1] = psml.tile(
                                [128, QS], F32, tag="sml", name="po1"
                            )
                        for p in ps:
                            nc.tensor.matmul(
                                po[eh][:],
                                ots[p][:, 128 * u : 128 * u + 128],
                                w2_all[:, p, QS * eh : QS * eh + QS],
                                start=(p == ps[0]),
                                stop=(p == ps[-1]),
                            )

                    return f

                def evac(u):
                    def f():
                        ob = osbp.tile([128, 2 * QS], BF, tag="osb", name="osb")
                        for eh in range(2):
                            nc.vector.tensor_copy(
                                ob[:, QS * eh : QS * eh + QS], po[eh][:]
                            )
                        d = dest if dest is not None else out_part[s]
                        if tail:
                            # tail half: nothing overlaps the stores, so
                            # spread them over three queues (exp is done,
                            # scalar is free) and split in halves
                            engs = (nc.sync, nc.gpsimd, nc.scalar)
                            for eh in range(2):
                                engs[(2 * u + eh) % 3].dma_start(
                                    d[
                                        128 * u : 128 * u + 128,
                                        QS * eh : QS * eh + QS,
                                    ],
                                    ob[:, QS * eh : QS * eh + QS],
                                )
                        else:
                            eng = nc.sync if u % 2 == 0 else nc.gpsimd
                            eng.dma_start(d[128 * u : 128 * u + 128], ob[:])

                    return f

                for u in range(4):
                    steps.append(mm(u, 0))
                    steps.append(mm(u, 1))
                    steps.append(evac(u))
                return steps

            # ---- pre-pulled scores+exp ----
            # the exp stream saturates the scalar engine in the late strips
            # (strip 3 has 27us of exp vs ~23us of PE); pull the first few
            # k-tiles' scores+exp of strip s into strip s-1, where the
            # scalar engine has slack.  The at tiles persist in dedicated
            # one-shot slots until the owning strip's attnV consumes them.
            ats_pre = {}
            PULLS = {1: (0, 1, 2, 3), 2: (0, 1, 2, 3), 3: (0, 1, 2, 3, 4, 5)}

            def pre_scores(s2, p, j):
                def f():
                    psc = pbig.tile(
                        [128, 2 * QS], F32, tag="big", name="pscp"
                    )
                    for hh in range(2):
                        hp = 64 * hh
                        nc.tensor.matmul(
                            psc[:, QS * hh : QS * hh + QS],
                            kt_sb[p][j // 4][
                                hp : hp + 64,
                                128 * (j % 4) : 128 * (j % 4) + 128,
                            ],
                            qt_sb[p][s2][hp : hp + 64, :],
                            start=True,
                            stop=True,
                        )
                    at = atf.tile(
                        [128, 2 * QS], BF, tag=f"pre{s2 % 2}{p}{j}",
                        name=f"pre{s2}{p}{j}",
                    )
                    nc.scalar.activation(at[:], psc[:], AF.Exp, scale=EXP_SCALE)
                    ats_pre[(s2, p, j)] = at

                return f

            # ---- main pipeline ----
            load_x(0)
            load_x(1)
            nc.gpsimd.dma_start(w2_all[:], w2[:])
            # strip 0: emit only the Q/K projection now.  The V matmuls
            # wait on the xv0 DMA (~11us) and would block the in-order PE
            # queue ahead of strip-0's first scores; they are emitted after
            # the scores prologue instead.
            p0_steps = proj_steps(0)
            n_qk0 = (4 if QK_FP8 else 8) * 2 + 2
            for st in p0_steps[:n_qk0]:
                st()
            v0_steps = p0_steps[n_qk0:]

            pending = []

            def pump(n):
                for _ in range(min(n, len(pending))):
                    pending.pop(0)()

            for s in range(NQS):
                if s + 2 < NQS:
                    pending.append(lambda s2=s + 2: load_x(s2))
                if s + 1 < NQS:
                    ps_list = proj_steps(s + 1)
                    pulls = [
                        pre_scores(s + 1, p2, j)
                        for j in PULLS.get(s + 1, ())
                        for p2 in range(2)
                    ]
                    # splice the pulled scores+exp evenly through the proj
                    # steps (all after q_evac, which they depend on) so the
                    # exp stream spreads across the strip instead of
                    # bunching at its end
                    n_qk = 4 if QK_FP8 else 8
                    qe = n_qk + 1  # index just past q_evac
                    body = ps_list[qe:]
                    merged = ps_list[:qe]
                    if pulls:
                        step = max(1, len(body) // len(pulls))
                        bi = 0
                        for k, pu in enumerate(pulls):
                            nxt = min(len(body), (k + 1) * step)
                            merged += body[bi:nxt]
                            merged.append(pu)
                            bi = nxt
                        merged += body[bi:]
                    else:
                        merged += body
                    pending += merged
                jmax = 4 * s + 4
                # interleave budget: spread pending steps over this strip's
                # (pair, j) iterations, skipping the first few so the
                # attention front isn't stalled by not-yet-landed x DMAs
                skip = 3 if s == 0 else 2
                iters = 2 * (jmax + 2) - skip
                rate = (len(pending) + 2 + iters - 1) // iters
                it_ctr = [0]

                ot = [
                    otp.tile([128, QS], BF, tag=f"ot{p}", name=f"ot{p}")
                    for p in range(2)
                ]
                for p in range(2):
                    pso = {}
                    ats = {}

                    def do_scores(j):
                        pre = ats_pre.pop((s, p, j), None)
                        if pre is not None:
                            ats[j] = pre
                            return
                        psc = pbig.tile(
                            [128, 2 * QS], F32, tag="big", name="psc"
                        )
                        i = j - 4 * s
                        # diagonal tiles: columns < 128*i are fully masked --
                        # don't even compute them
                        off = 128 * i if j >= 4 * s else 0
                        for hh in range(2):
                            hp = 64 * hh
                            nc.tensor.matmul(
                                psc[:, QS * hh + off : QS * hh + QS],
                                kt_sb[p][j // 4][
                                    hp : hp + 64, 128 * (j % 4) : 128 * (j % 4) + 128
                                ],
                                qt_sb[p][s][hp : hp + 64, off:QS],
                                start=True,
                                stop=True,
                            )
                        if j >= 4 * s:
                            # causal mask: accumulate -1e5 onto the masked
                            # cells of the 128-wide diagonal block (id.T @
                            # trineg = trineg), so exp zeroes them with no
                            # DVE op in the attnV critical path
                            for hh in range(2):
                                o = QS * hh + off
                                nc.tensor.matmul(
                                    psc[:, o : o + 128],
                                    id_sb[:],
                                    tneg_sb[:],
                                    start=False,
                                    stop=True,
                                    skip_group_check=True,
                                )
                        at = atp.tile([128, 2 * QS], BF, tag="at", name="at")
                        if off == 0:
                            nc.scalar.activation(
                                at[:], psc[:], AF.Exp, scale=EXP_SCALE
                            )
                        else:
                            # one strided ACT covers both head-halves'
                            # unmasked columns
                            sl = (
                                lambda t: t.rearrange(
                                    "pp (h c) -> pp h c", h=2
                                )[:, :, off:QS]
                            )
                            nc.scalar.activation(
                                sl(at[:]), sl(psc[:]), AF.Exp,
                                scale=EXP_SCALE,
                            )
                        ats[j] = at

                    def attn_v(hh, j):
                        off = 128 * (j - 4 * s) if j >= 4 * s else 0
                        if j == 0:
                            pso[hh] = ppso.tile(
                                [DH + 1, QS], F32, tag="pso", name=f"pso{hh}"
                            )
                        nc.tensor.matmul(
                            pso[hh][:, off:QS],
                            v_sb[j][:, 2 * p + hh, :],
                            ats[j][:, QS * hh + off : QS * hh + QS],
                            start=(j == 0),
                            stop=(j == jmax - 1),
                        )

                    # normalize: rowsum (psum row DH) -> reciprocal ->
                    # multiply into the persistent OT tile.  Split in two:
                    # the pre half only issues the DVE rowsum copy; the PE
                    # half (rbc broadcast) is emitted later with PE filler
                    # in between so the in-order PE queue never waits on
                    # the DVE queue draining (that wait was >3.4us and
                    # re-throttled the HAM clock at every strip boundary).
                    rss = {}

                    def normalize_pre(hh):
                        rs = nrm.tile([1, QS], BF, tag="rs", name="rs")
                        nc.vector.tensor_copy(rs[:], pso[hh][DH : DH + 1])
                        rss[hh] = rs

                    def normalize_post(hh):
                        rbc = pbig.tile([64, QS], F32, tag="big", name="rbc")
                        nc.tensor.matmul(
                            rbc[:], ones_sb[:], rss[hh][:], start=True,
                            stop=True,
                        )
                        rrec = nrm.tile([64, QS], F32, tag="rrec", name="rrec")
                        nc.vector.reciprocal_approx_fast(rrec[:], rbc[:])
                        nc.vector.tensor_tensor(
                            ot[p][64 * hh : 64 * hh + 64],
                            pso[hh][0:DH],
                            rrec[:],
                            ALU.mult,
                        )

                    # hh1's attnV stream lags hh0 by two k-tiles so each
                    # head-half's normalize chain overlaps remaining matmuls
                    # and its pso bank frees before the next pair needs it
                    # scores cursor: keep TWO not-yet-pulled score tiles in
                    # flight ahead of attnV.  Pulled tiles are free dict
                    # hits, so during a pulled window the cursor streams
                    # real scores (and their exps) ahead -- this is what
                    # keeps the scalar engine fed at strip starts.
                    npull = sum(1 for jj in PULLS.get(s, ()) if jj < jmax)
                    cur = [0]

                    def emit_scores_to(k):
                        while cur[0] < min(k, jmax):
                            do_scores(cur[0])
                            cur[0] += 1

                    emit_scores_to(npull + 2)
                    if s == 0 and p == 0:
                        # the first attnV waits ~15us for the xv0 DMA; the
                        # in-order PE queue would idle behind it, so slot
                        # strip-1's Q/K projection matmuls and the deferred
                        # strip-0 V projection in ahead of it
                        pump(12)
                        for st in v0_steps:
                            st()
                    for j in range(jmax + 2):
                        emit_scores_to(j + 3)
                        it_ctr[0] += 1
                        if it_ctr[0] > skip:
                            pump(rate)
                        if j < jmax:
                            attn_v(0, j)
                            if j == jmax - 1:
                                normalize_pre(0)
                        if j >= 2:
                            attn_v(1, j - 2)
                            if j - 2 == jmax - 1:
                                normalize_pre(1)
                            del ats[j - 2]
                        if j == jmax:
                            normalize_post(0)
                    if p == 1:
                        pump(len(pending))
                    normalize_post(1)
                    if s == NQS - 1 and p == 0:
                        # last strip: pair-0's half of the out-projection
                        # goes through `pending` so it interleaves with
                        # pair-1's attention instead of sitting in the tail
                        pending.extend(
                            outproj_steps(s, ot, ps=(0,), dest=out_extra)
                        )
                # out-projection of this strip: route it through `pending`
                # so its matmuls AND its DVE casts spread across the next
                # strip's attention instead of forming a block at the strip
                # boundary (the DVE FIFO block was stalling the PE)
                if s == NQS - 1:
                    pump(len(pending))
                    for st in outproj_steps(
                        s, ot, ps=(1,), dest=out_part[s], tail=True
                    ):
                        st()
                else:
                    pending.extend(outproj_steps(s, ot))

    nc.compile()
    return nc


_NC = None
_RUNNER = None


def _get_runner():
    """Build the compiled 8-core PJRT callable once and cache it."""
    global _NC, _RUNNER
    if _RUNNER is not None:
        return _RUNNER

    import jax
    import numpy as _np
    from jax.sharding import Mesh, PartitionSpec
    from jax.experimental.shard_map import shard_map
    from concourse.bass2jax import (
        _bass_exec_p,
        install_neuronx_cc_hook,
        partition_id_tensor,
    )

    _NC = build_nc()
    nc = _NC
    install_neuronx_cc_hook()

    partition_name = nc.partition_id_tensor.name if nc.partition_id_tensor else None
    in_names = []
    out_names = []
    out_avals = []
    zero_outs = []
    for alloc in nc.m.functions[0].allocations:
        if not isinstance(alloc, mybir.MemoryLocationSet):
            continue
        name = alloc.memorylocations[0].name
        if alloc.kind == "ExternalInput":
            if name != partition_name:
                in_names.append(name)
        elif alloc.kind == "ExternalOutput":
            shape = tuple(alloc.tensor_shape)
            dtype = mybir.dt.np(alloc.dtype)
            out_names.append(name)
            out_avals.append(jax.core.ShapedArray(shape, dtype))
            zero_outs.append(_np.zeros(shape, dtype))
    n_params = len(in_names)
    n_outs = len(out_avals)
    all_in_names = list(in_names) + list(out_names)
    if partition_name is not None:
        all_in_names.append(partition_name)

    def _body(*args):
        operands = list(args)
        if partition_name is not None:
            operands.append(partition_id_tensor())
        outs = _bass_exec_p.bind(
            *operands,
            out_avals=tuple(out_avals),
            in_names=tuple(all_in_names),
            out_names=tuple(out_names),
            lowering_input_output_aliases=(),
            sim_require_finite=True,
            sim_require_nnan=True,
            nc=nc,
        )
        return tuple(outs)

    devices = jax.devices()[:N_CORES]
    mesh = Mesh(np.asarray(devices), ("core",))
    in_specs = (PartitionSpec("core"),) * (n_params + n_outs)
    out_specs = (PartitionSpec("core"),) * n_outs
    sharded = jax.jit(
        shard_map(
            _body, mesh=mesh, in_specs=in_specs, out_specs=out_specs, check_rep=False
        ),
        keep_unused=True,
    )

    def run(in_maps):
        per_core = [[_np.asarray(m[name]) for name in in_names] for m in in_maps]
        concat_in = [
            _np.concatenate([per_core[c][i] for c in range(N_CORES)], axis=0)
            for i in range(n_params)
        ]
        concat_zeros = [
            _np.zeros((N_CORES * z.shape[0], *z.shape[1:]), z.dtype)
            for z in zero_outs
        ]
        out_arrs = sharded(*concat_in, *concat_zeros)
        return [
            {
                name: _np.asarray(out_arrs[i]).reshape(
                    N_CORES, *out_avals[i].shape
                )[c]
                for i, name in enumerate(out_names)
            }
            for c in range(N_CORES)
        ]

    _RUNNER = run
    return run


_BO_EFF = None


def make_in_maps(query, key, value, Wq, bq, Wk, bk, Wv, bv, Wo, bo):
    global _BO_EFF
    from ml_dtypes import bfloat16, float8_e4m3

    query = np.asarray(query, dtype=np.float32)
    key = np.asarray(key, dtype=np.float32)
    value = np.asarray(value, dtype=np.float32)
    Wq = np.asarray(Wq, dtype=np.float32)
    bq = np.asarray(bq, dtype=np.float32)
    Wk = np.asarray(Wk, dtype=np.float32)
    Wv = np.asarray(Wv, dtype=np.float32)
    bv = np.asarray(bv, dtype=np.float32)
    Wo = np.asarray(Wo, dtype=np.float32)
    bo = np.asarray(bo, dtype=np.float32)

    # K bias is softmax-invariant (constant per q row) -> dropped.
    # V bias: softmax rows sum to 1, so it contributes bv @ Wo.T -> fold
    # into the output bias, added on host during assembly.
    _BO_EFF = bo + bv @ Wo.T

    xqk_np = float8_e4m3 if QK_FP8 else bfloat16
    # Wq/Wk entries are ~U(-1/32, 1/32) -- scale by 8 so every value is
    # fp8-normal; Q,K come out scaled by 8 and exp's scale absorbs it
    wqk_scale = 8.0 if QK_FP8 else 1.0

    # x strip-major: xt[s, p, t, c] = x[512 s + c, 128 t + p]
    def tile_x(x, dt):  # [S, HID] -> [NQS, 128, 8, QS]
        t = x.reshape(NQS, QS, 8, 128).transpose(0, 3, 2, 1)
        return np.ascontiguousarray(t).astype(dt)

    xqs = [tile_x(query[b], xqk_np) for b in range(B)]
    xks = [tile_x(key[b], xqk_np) for b in range(B)]
    xvs = [tile_x(value[b], bfloat16) for b in range(B)]

    # causal-mask helpers for the diagonal 128x128 block: identity (matmul
    # lhsT) and -1e5 on strictly-masked cells [k, q] (q < k)
    idm = np.ascontiguousarray(np.eye(128, dtype=bfloat16))
    trineg = np.ascontiguousarray(
        np.where(
            np.arange(128)[None, :] < np.arange(128)[:, None], -1.0e5, 0.0
        ).astype(bfloat16)
    )

    def tile_w(wT, dt):  # [HID, F] (= W[hsl].T) -> [128, 8, F]
        t = wT.reshape(8, 128, -1).transpose(1, 0, 2)
        return np.ascontiguousarray(t).astype(dt)

    in_maps = []
    for c in range(N_CORES):
        b = c // GROUP
        g = c % GROUP
        hsl = slice(HG * g, HG * g + HG)
        wq_g = tile_w(Wq[hsl].T * wqk_scale, xqk_np)  # [128, 8, 256]
        wk_g = tile_w(Wk[hsl].T * wqk_scale, xqk_np)
        wv_g = tile_w(Wv[hsl].T, bfloat16)
        w2_t = Wo[:, hsl].T.reshape(2, 128, HID).transpose(1, 0, 2)
        w2_g = np.ascontiguousarray(w2_t).astype(bfloat16)  # [128, 2, 1024]
        # Q bias rides on the 8x-scaled Q
        bq_g = np.ascontiguousarray(
            bq[hsl].reshape(2, 128).T * wqk_scale
        ).astype(np.float32)
        in_maps.append(
            {
                "xq": xqs[b],
                "xk": xks[b],
                "xv": xvs[b],
                "wq": wq_g,
                "wk": wk_g,
                "wv": wv_g,
                "w2": w2_g,
                "bqv": bq_g,
                "idm": idm,
                "trineg": trineg,
            }
        )
    return in_maps


def assemble_output(results):
    # core group {4b..4b+3} holds bf16 partial out-projections of batch b;
    # sum them (the row-parallel TP unshard) and add the folded bias
    out = np.empty((B, S, HID), dtype=np.float32)
    for b in range(B):
        acc = results[GROUP * b]["out_part"].astype(np.float32)
        acc[NQS - 1] += results[GROUP * b]["out_extra"].astype(np.float32)
        for r in range(1, GROUP):
            res = results[GROUP * b + r]
            acc = acc + res["out_part"].astype(np.float32)
            acc[NQS - 1] += res["out_extra"].astype(np.float32)
        out[b] = acc.reshape(S, HID)
    out += _BO_EFF
    return out


def kernel(**inputs) -> np.ndarray:
    in_maps = make_in_maps(**inputs)
    run = _get_runner()
    results = run(in_maps)
    return assemble_output(results)


if __name__ == "__main__":
    import reference

    inputs = {k: np.asarray(v) for k, v in reference.setup_inputs().items()}
    got = kernel(**inputs)
    want = np.asarray(reference.reference(**inputs))
    err = np.linalg.norm(got - want) / np.linalg.norm(want)
    print("Relative error:", err)
